# revision 1
# baseline (speedup 1.0000x reference)
"""Trainium2 Bass kernel for a dense transformer block (nn_Block_65987877535901).

Strategy: collective-free sequence-parallel sharding over 8 NeuronCores.
Core c handles query tokens [512*(c%4), 512*(c%4+1)) of batch c//4.
Each core computes K/V for its full batch (2048 tokens, replicated within
the 4-core batch group), full causal attention for its 512 queries with a
host-uploaded mask, then proj / LN2 / MLP purely token-locally.  The host
concatenates the 8 per-core outputs — zero device-to-device communication.

Layout: feature-major activations [dim, tokens] throughout, so all weights
are used in their natural [in, out] layout with no transposes.  LayerNorm
reductions over the feature (partition) dim are done with ones-vector
matmuls on the TensorEngine; per-token stats are broadcast across
partitions with stride-0 DMA.  Attention scores are computed transposed
(S^T = K_fm^T Q_fm, i.e. keys on partitions) so softmax needs no
transposes: exp without max-subtraction (scores are O(5)), causal mask as
a multiplicative bf16 mask after exp, and normalization deferred to a
per-head column scale of y using a row of ones appended to V to produce
row-sums during the att@V matmul.
"""
import sys

sys.path.insert(0, "/opt/trn_rl_repo")

import numpy as np
import ml_dtypes

BF16 = ml_dtypes.bfloat16

P = 128
C = 1024          # embed dim -> 8 chunks
NCH = C // P      # 8
T = 2048          # tokens per batch (kv length)
NT = 512          # own query tokens per core
H = 16            # heads
D = 64            # head dim
F = 4096          # mlp hidden -> 32 chunks
NFCH = F // P     # 32
NKT = T // P      # 16 kv tiles
NCORES = 8
EPS = 1e-5

_COMPILED = None


def _build():
    import concourse.bacc as bacc
    import concourse.tile as tile
    import concourse.bass as bass
    from concourse import mybir

    dt = mybir.dt
    BF = dt.bfloat16
    F32 = dt.float32
    AF = mybir.ActivationFunctionType
    OP = mybir.AluOpType

    nc = bacc.Bacc("TRN2", target_bir_lowering=False, debug=False,
                   num_devices=NCORES)

    # ---- kernel I/O ----
    xkv = nc.declare_dram_parameter("xkv", [C, T], BF, isOutput=False)
    xq = nc.declare_dram_parameter("xq", [C, NT], BF, isOutput=False)
    xres = nc.declare_dram_parameter("xres", [C, NT], F32, isOutput=False)
    maskd = nc.declare_dram_parameter("maskd", [NKT, P, NT], BF, isOutput=False)
    wq = nc.declare_dram_parameter("wq", [C, C], BF, isOutput=False)
    wk = nc.declare_dram_parameter("wk", [C, C], BF, isOutput=False)
    wv = nc.declare_dram_parameter("wv", [C, C], BF, isOutput=False)
    wp = nc.declare_dram_parameter("wp", [C, C], BF, isOutput=False)
    wfc = nc.declare_dram_parameter("wfc", [C, F], BF, isOutput=False)
    wfc2 = nc.declare_dram_parameter("wfc2", [F, C], BF, isOutput=False)
    bq8 = nc.declare_dram_parameter("bq8", [C], F32, isOutput=False)
    bk = nc.declare_dram_parameter("bk", [C], F32, isOutput=False)
    bv = nc.declare_dram_parameter("bv", [C], F32, isOutput=False)
    bp = nc.declare_dram_parameter("bp", [C], F32, isOutput=False)
    bfc = nc.declare_dram_parameter("bfc", [F], F32, isOutput=False)
    bfc2 = nc.declare_dram_parameter("bfc2", [C], F32, isOutput=False)
    g1 = nc.declare_dram_parameter("g1", [C], F32, isOutput=False)
    b1 = nc.declare_dram_parameter("b1", [C], F32, isOutput=False)
    g2 = nc.declare_dram_parameter("g2", [C], F32, isOutput=False)
    b2 = nc.declare_dram_parameter("b2", [C], F32, isOutput=False)
    out = nc.declare_dram_parameter("out", [C, NT], F32, isOutput=True)

    def bcast_ap(src, nparts):
        """stride-0 partition broadcast view of a [1, N] AP"""
        return bass.AP(tensor=src.tensor, offset=src.offset,
                       ap=[[0, nparts]] + list(src.ap[-1:]))

    def chunk_col_ap(dram, nchunks):
        """[nchunks*P] dram vector viewed as [P, nchunks] (partition-major)"""
        a = dram.ap()
        return bass.AP(tensor=a.tensor, offset=a.offset,
                       ap=[[1, P], [P, nchunks]])

    with tile.TileContext(nc) as tc:
        with (
            tc.tile_pool(name="const", bufs=1) as const,
            tc.tile_pool(name="persist", bufs=1) as persist,
            tc.tile_pool(name="dramp", bufs=4, space="DRAM") as dramp,
        ):
            def bcast_rt(dst, src_1xn, dtype):
                scr = dramp.tile([1, src_1xn.shape[-1]], dtype, tag="scr",
                                 name="scr")
                nc.sync.dma_start(scr[:], src_1xn)
                nc.sync.dma_start(dst, bcast_ap(scr[:], dst.shape[0]))

            ones1 = const.tile([P, 1], BF, tag="ones1", name="ones1")
            nc.vector.memset(ones1[:], 1.0)
            epst = const.tile([1, 1], F32, tag="epst", name="epst")
            nc.vector.memset(epst[:], EPS)

            bias_q = const.tile([P, NCH], F32, tag="bias_q", name="bias_q")
            nc.sync.dma_start(bias_q[:], chunk_col_ap(bq8, NCH))
            bias_k = const.tile([P, NCH], F32, tag="bias_k", name="bias_k")
            nc.sync.dma_start(bias_k[:], chunk_col_ap(bk, NCH))
            bias_p = const.tile([P, NCH], F32, tag="bias_p", name="bias_p")
            nc.sync.dma_start(bias_p[:], chunk_col_ap(bp, NCH))
            bias_fc = const.tile([P, NFCH], F32, tag="bias_fc", name="bias_fc")
            nc.sync.dma_start(bias_fc[:], chunk_col_ap(bfc, NFCH))
            bias_fc2 = const.tile([P, NCH], F32, tag="bias_fc2", name="bias_fc2")
            nc.sync.dma_start(bias_fc2[:], chunk_col_ap(bfc2, NCH))
            g1t = const.tile([P, NCH], F32, tag="g1t", name="g1t")
            nc.sync.dma_start(g1t[:], chunk_col_ap(g1, NCH))
            b1t = const.tile([P, NCH], F32, tag="b1t", name="b1t")
            nc.sync.dma_start(b1t[:], chunk_col_ap(b1, NCH))
            g2t = const.tile([P, NCH], F32, tag="g2t", name="g2t")
            nc.sync.dma_start(g2t[:], chunk_col_ap(g2, NCH))
            b2t = const.tile([P, NCH], F32, tag="b2t", name="b2t")
            nc.sync.dma_start(b2t[:], chunk_col_ap(b2, NCH))
            bvbc = const.tile([P, C], F32, tag="bvbc", name="bvbc")
            nc.sync.dma_start(
                bvbc[:],
                bass.AP(tensor=bv.ap().tensor, offset=bv.ap().offset,
                        ap=[[0, P], [1, C]]))

            # persistent activations
            kf = [persist.tile([P, T], BF, tag=f"kf{m}", name=f"kf{m}") for m in range(NCH)]
            vaug = [persist.tile([P, H * (D + 1)], BF, tag=f"va{t}",
                                 name=f"va{t}")
                    for t in range(NKT)]
            qf = [persist.tile([P, NT], BF, tag=f"qf{m}", name=f"qf{m}") for m in range(NCH)]
            yf = [persist.tile([P, NT], BF, tag=f"yf{m}", name=f"yf{m}") for m in range(NCH)]
            h1 = [persist.tile([P, NT], F32, tag=f"h1{m}", name=f"h1{m}") for m in range(NCH)]

            def layernorm_fm(src_dma, ntok, gt, bt, dst_tiles, tag):
                """LayerNorm over the feature (partition) dim of a
                feature-major [C, ntok] DRAM tensor; writes bf16
                normalized chunks into dst_tiles ([P, ntok] x NCH).
                src_dma(i) -> DRAM AP of feature chunk i."""
                nsl = ntok // NT
                with (
                    tc.tile_pool(name=f"ln_{tag}", bufs=2) as lnp,
                    tc.tile_pool(name=f"lns_{tag}", bufs=1) as lns,
                    tc.tile_pool(name=f"lnps_{tag}", bufs=1,
                                 space="PSUM") as lnps,
                ):
                    stp = lnps.tile([1, nsl * 1024], F32, tag="st", name="st")
                    # pass A: stats
                    for i in range(NCH):
                        xi = lnp.tile([P, ntok], BF, tag="xi", name="xi")
                        nc.sync.dma_start(xi[:], src_dma(i))
                        sqi = lnp.tile([P, ntok], BF, tag="sqi", name="sqi")
                        nc.vector.tensor_mul(sqi[:], xi[:], xi[:])
                        for sl in range(nsl):
                            nc.tensor.matmul(
                                stp[0:1, sl * 1024:sl * 1024 + 512],
                                ones1[:], xi[:, sl * NT:(sl + 1) * NT],
                                start=(i == 0), stop=(i == NCH - 1))
                            nc.tensor.matmul(
                                stp[0:1, sl * 1024 + 512:sl * 1024 + 1024],
                                ones1[:], sqi[:, sl * NT:(sl + 1) * NT],
                                start=(i == 0), stop=(i == NCH - 1))
                    # stats -> A/B broadcast tiles
                    abc = []
                    for sl in range(nsl):
                        mu = lnp.tile([1, NT], F32, tag="mu", name="mu")
                        nc.vector.tensor_scalar_mul(
                            mu[:], stp[0:1, sl * 1024:sl * 1024 + 512],
                            1.0 / C)
                        var = lnp.tile([1, NT], F32, tag="var", name="var")
                        nc.vector.tensor_scalar_mul(
                            var[:],
                            stp[0:1, sl * 1024 + 512:sl * 1024 + 1024],
                            1.0 / C)
                        musq = lnp.tile([1, NT], F32, tag="musq", name="musq")
                        nc.vector.tensor_mul(musq[:], mu[:], mu[:])
                        nc.vector.tensor_sub(var[:], var[:], musq[:])
                        std = lnp.tile([1, NT], F32, tag="std", name="std")
                        nc.scalar.activation(std[:], var[:], AF.Sqrt,
                                             bias=epst[:])
                        af = lnp.tile([1, NT], F32, tag="af", name="af")
                        nc.vector.reciprocal(af[:], std[:])
                        bf = lnp.tile([1, NT], F32, tag="bfl", name="bfl")
                        nc.vector.tensor_mul(bf[:], mu[:], af[:])
                        ab16 = lnp.tile([1, NT], BF, tag="ab16", name="ab16")
                        nc.vector.tensor_copy(ab16[:], af[:])
                        bb16 = lnp.tile([1, NT], BF, tag="bb16", name="bb16")
                        nc.vector.tensor_copy(bb16[:], bf[:])
                        abc_t = lns.tile([P, NT], BF, tag=f"abc{sl}", name=f"abc{sl}")
                        bcast_rt(abc_t[:], ab16[:], BF)
                        bbc_t = lns.tile([P, NT], BF, tag=f"bbc{sl}", name=f"bbc{sl}")
                        bcast_rt(bbc_t[:], bb16[:], BF)
                        abc.append((abc_t, bbc_t))
                    # pass B: normalize
                    for i in range(NCH):
                        xi = lnp.tile([P, ntok], BF, tag="xi", name="xi")
                        nc.sync.dma_start(xi[:], src_dma(i))
                        for sl in range(nsl):
                            at, btl = abc[sl]
                            tt = lnp.tile([P, NT], BF, tag="tt", name="tt")
                            nc.vector.tensor_mul(
                                tt[:], xi[:, sl * NT:(sl + 1) * NT], at[:])
                            nc.vector.tensor_sub(tt[:], tt[:], btl[:])
                            nc.vector.tensor_scalar(
                                dst_tiles[i][:, sl * NT:(sl + 1) * NT],
                                tt[:], gt[:, i:i + 1], bt[:, i:i + 1],
                                OP.mult, OP.add)

            # ================= phase 1: LN1 =================
            with tc.tile_pool(name="xn1p", bufs=1) as xn1p:
                xn1 = [xn1p.tile([P, T], BF, tag=f"xn1_{i}", name=f"xn1_{i}")
                       for i in range(NCH)]
                layernorm_fm(lambda i: xkv[i * P:(i + 1) * P, :], T,
                             g1t, b1t, xn1, "kv")
                xnq = [xn1p.tile([P, NT], BF, tag=f"xnq_{i}", name=f"xnq_{i}")
                       for i in range(NCH)]
                layernorm_fm(lambda i: xq[i * P:(i + 1) * P, :], NT,
                             g1t, b1t, xnq, "q")

                # ================= phase 2: QKV =================
                with (
                    tc.tile_pool(name="wqkv", bufs=1) as wpool,
                    tc.tile_pool(name="ps2", bufs=4, space="PSUM") as ps2,
                    tc.tile_pool(name="ps2v", bufs=2, space="PSUM") as ps2v,
                ):
                    wkt = [wpool.tile([P, C], BF, tag=f"wk{k}", name=f"wk{k}")
                           for k in range(NCH)]
                    wvt = [wpool.tile([P, C], BF, tag=f"wv{k}", name=f"wv{k}")
                           for k in range(NCH)]
                    wqt = [wpool.tile([P, C], BF, tag=f"wq{k}", name=f"wq{k}")
                           for k in range(NCH)]
                    for k in range(NCH):
                        nc.sync.dma_start(wkt[k][:], wk[k * P:(k + 1) * P, :])
                        nc.sync.dma_start(wvt[k][:], wv[k * P:(k + 1) * P, :])
                        nc.sync.dma_start(wqt[k][:], wq[k * P:(k + 1) * P, :])
                    # K feature-major
                    for m in range(NCH):
                        for n in range(T // NT):
                            ps = ps2.tile([P, NT], F32, tag="kq", name="kq")
                            for k in range(NCH):
                                nc.tensor.matmul(
                                    ps[:], wkt[k][:, m * P:(m + 1) * P],
                                    xn1[k][:, n * NT:(n + 1) * NT],
                                    start=(k == 0), stop=(k == NCH - 1))
                            nc.scalar.activation(
                                kf[m][:, n * NT:(n + 1) * NT], ps[:],
                                AF.Identity, bias=bias_k[:, m:m + 1])
                    # Q feature-major (pre-scaled by 1/8)
                    for m in range(NCH):
                        ps = ps2.tile([P, NT], F32, tag="kq", name="kq")
                        for k in range(NCH):
                            nc.tensor.matmul(
                                ps[:], wqt[k][:, m * P:(m + 1) * P],
                                xnq[k][:], start=(k == 0),
                                stop=(k == NCH - 1))
                        nc.scalar.activation(
                            qf[m][:], ps[:], AF.Identity,
                            bias=bias_q[:, m:m + 1], scale=0.125)
                    # V token-major, head-interleaved with ones column
                    for t in range(NKT):
                        psv = ps2v.tile([P, C], F32, tag="v", name="v")
                        for half in range(2):
                            for k in range(NCH):
                                nc.tensor.matmul(
                                    psv[:, half * 512:(half + 1) * 512],
                                    xn1[k][:, t * P:(t + 1) * P],
                                    wvt[k][:, half * 512:(half + 1) * 512],
                                    start=(k == 0), stop=(k == NCH - 1))
                        v3 = vaug[t].rearrange("p (h x) -> p h x", h=H)
                        nc.vector.scalar_tensor_tensor(
                            v3[:, :, 0:D], psv[:].rearrange(
                                "p (h x) -> p h x", h=H),
                            1.0, bvbc[:].rearrange("p (h x) -> p h x", h=H),
                            OP.mult, OP.add)
                        nc.vector.memset(v3[:, :, D:D + 1], 1.0)

            # ================= phase 3: attention =================
            with (
                tc.tile_pool(name="attn", bufs=1) as attnp,
                tc.tile_pool(name="epool", bufs=3) as epool,
                tc.tile_pool(name="rpool", bufs=2) as rpool,
                tc.tile_pool(name="pse", bufs=2, space="PSUM") as pse,
                tc.tile_pool(name="psy", bufs=2, space="PSUM") as psyp,
            ):
                maskt = attnp.tile([P, NKT * NT], BF, tag="mask", name="mask")
                nc.sync.dma_start(
                    maskt[:].rearrange("p (a b) -> p a b", a=NKT),
                    bass.AP(tensor=maskd.ap().tensor,
                            offset=maskd.ap().offset,
                            ap=[[NT, P], [P * NT, NKT], [1, NT]]))
                for hp in range(NCH):  # head pair = chunk hp
                    psy = [psyp.tile([D + 1, NT], F32, tag=f"psy{u}", name=f"psy{u}")
                           for u in range(2)]
                    for kg in range(NKT // 2):
                        pss = [pse.tile([P, 2 * NT], F32, tag="pse", name="pse")
                               for _ in range(2)]
                        for u in range(2):
                            for j in range(2):
                                kt = 2 * kg + j
                                nc.tensor.matmul(
                                    pss[u][:, j * NT:(j + 1) * NT],
                                    kf[hp][u * D:(u + 1) * D,
                                           kt * P:(kt + 1) * P],
                                    qf[hp][u * D:(u + 1) * D, :],
                                    start=True, stop=True)
                        for u in range(2):
                            et = epool.tile([P, 2 * NT], BF, tag="e", name="e")
                            nc.scalar.activation(et[:], pss[u][:], AF.Exp)
                            nc.vector.tensor_mul(
                                et[:], et[:],
                                maskt[:, kg * 2 * NT:(kg + 1) * 2 * NT])
                            for j in range(2):
                                kt = 2 * kg + j
                                h = 2 * hp + u
                                nc.tensor.matmul(
                                    psy[u][:],
                                    vaug[kt][:, h * (D + 1):
                                             (h + 1) * (D + 1)],
                                    et[:, j * NT:(j + 1) * NT],
                                    start=(kg == 0 and j == 0),
                                    stop=(kg == NKT // 2 - 1 and j == 1))
                    for u in range(2):
                        r = rpool.tile([1, NT], F32, tag="r", name="r")
                        nc.vector.reciprocal(r[:], psy[u][D:D + 1, :])
                        rbc = rpool.tile([D, NT], F32, tag="rbc", name="rbc")
                        bcast_rt(rbc[:], r[:], F32)
                        nc.vector.tensor_mul(
                            yf[hp][u * D:(u + 1) * D, :],
                            psy[u][0:D, :], rbc[:])

            # ================= phase 4: proj + residual =================
            with (
                tc.tile_pool(name="projw", bufs=1) as pjp,
                tc.tile_pool(name="ps4", bufs=4, space="PSUM") as ps4,
            ):
                wpt = [pjp.tile([P, C], BF, tag=f"wp{k}", name=f"wp{k}")
                       for k in range(NCH)]
                for k in range(NCH):
                    nc.sync.dma_start(wpt[k][:], wp[k * P:(k + 1) * P, :])
                xrt = [pjp.tile([P, NT], F32, tag=f"xr{m}", name=f"xr{m}")
                       for m in range(NCH)]
                for m in range(NCH):
                    nc.sync.dma_start(xrt[m][:], xres[m * P:(m + 1) * P, :])
                for m in range(NCH):
                    ps = ps4.tile([P, NT], F32, tag="pj", name="pj")
                    for k in range(NCH):
                        nc.tensor.matmul(
                            ps[:], wpt[k][:, m * P:(m + 1) * P], yf[k][:],
                            start=(k == 0), stop=(k == NCH - 1))
                    nc.vector.scalar_tensor_tensor(
                        h1[m][:], ps[:], bias_p[:, m:m + 1], xrt[m][:],
                        OP.add, OP.add)

            # ================= phase 5: LN2 + MLP =================
            with tc.tile_pool(name="xn2p", bufs=1) as xn2p:
                # LN2 on h1 (feature-major, ones-matmul stats)
                xn2 = [xn2p.tile([P, NT], BF, tag=f"xn2_{i}", name=f"xn2_{i}")
                       for i in range(NCH)]
                h1b = [xn2p.tile([P, NT], BF, tag=f"h1b{i}", name=f"h1b{i}")
                       for i in range(NCH)]
                with (
                    tc.tile_pool(name="ln2w", bufs=2) as lnp,
                    tc.tile_pool(name="ln2ps", bufs=1, space="PSUM") as lnps,
                ):
                    stp = lnps.tile([1, 1024], F32, tag="st2", name="st2")
                    for i in range(NCH):
                        nc.scalar.activation(h1b[i][:], h1[i][:], AF.Copy)
                        sqi = lnp.tile([P, NT], BF, tag="sq2", name="sq2")
                        nc.vector.tensor_mul(sqi[:], h1b[i][:], h1b[i][:])
                        nc.tensor.matmul(stp[0:1, 0:512], ones1[:],
                                         h1b[i][:], start=(i == 0),
                                         stop=(i == NCH - 1))
                        nc.tensor.matmul(stp[0:1, 512:1024], ones1[:],
                                         sqi[:], start=(i == 0),
                                         stop=(i == NCH - 1))
                    mu = lnp.tile([1, NT], F32, tag="mu2", name="mu2")
                    nc.vector.tensor_scalar_mul(mu[:], stp[0:1, 0:512],
                                                1.0 / C)
                    var = lnp.tile([1, NT], F32, tag="var2", name="var2")
                    nc.vector.tensor_scalar_mul(var[:], stp[0:1, 512:1024],
                                                1.0 / C)
                    musq = lnp.tile([1, NT], F32, tag="musq2", name="musq2")
                    nc.vector.tensor_mul(musq[:], mu[:], mu[:])
                    nc.vector.tensor_sub(var[:], var[:], musq[:])
                    std = lnp.tile([1, NT], F32, tag="std2", name="std2")
                    nc.scalar.activation(std[:], var[:], AF.Sqrt, bias=epst[:])
                    af = lnp.tile([1, NT], F32, tag="af2", name="af2")
                    nc.vector.reciprocal(af[:], std[:])
                    bfl = lnp.tile([1, NT], F32, tag="bfl2", name="bfl2")
                    nc.vector.tensor_mul(bfl[:], mu[:], af[:])
                    ab16 = lnp.tile([1, NT], BF, tag="a216", name="a216")
                    nc.vector.tensor_copy(ab16[:], af[:])
                    bb16 = lnp.tile([1, NT], BF, tag="b216", name="b216")
                    nc.vector.tensor_copy(bb16[:], bfl[:])
                    abc_t = lnp.tile([P, NT], BF, tag="abc2", name="abc2")
                    bcast_rt(abc_t[:], ab16[:], BF)
                    bbc_t = lnp.tile([P, NT], BF, tag="bbc2", name="bbc2")
                    bcast_rt(bbc_t[:], bb16[:], BF)
                    for i in range(NCH):
                        tt = lnp.tile([P, NT], BF, tag="tt2", name="tt2")
                        nc.vector.tensor_mul(tt[:], h1b[i][:], abc_t[:])
                        nc.vector.tensor_sub(tt[:], tt[:], bbc_t[:])
                        nc.vector.tensor_scalar(
                            xn2[i][:], tt[:], g2t[:, i:i + 1],
                            b2t[:, i:i + 1], OP.mult, OP.add)

                # MLP fc + gelu
                with tc.tile_pool(name="hmlpp", bufs=1) as hmlpp:
                    hmlp = [hmlpp.tile([P, NT], BF, tag=f"hm{m}", name=f"hm{m}")
                            for m in range(NFCH)]
                    with (
                        tc.tile_pool(name="wfcp", bufs=3) as wfcp,
                        tc.tile_pool(name="ps6", bufs=4, space="PSUM") as ps6,
                    ):
                      for mg in range(NFCH // 4):
                          pss = [ps6.tile([P, NT], F32, tag="fc", name="fc")
                                 for _ in range(4)]
                          for k in range(NCH):
                              wt = wfcp.tile([P, 4 * P], BF, tag="wfc", name="wfc")
                              nc.sync.dma_start(
                                  wt[:],
                                  wfc[k * P:(k + 1) * P,
                                      mg * 4 * P:(mg + 1) * 4 * P])
                              for mm in range(4):
                                  nc.tensor.matmul(
                                      pss[mm][:], wt[:, mm * P:(mm + 1) * P],
                                      xn2[k][:], start=(k == 0),
                                      stop=(k == NCH - 1))
                          for mm in range(4):
                              m = 4 * mg + mm
                              nc.scalar.activation(
                                  hmlp[m][:], pss[mm][:], AF.Gelu,
                                  bias=bias_fc[:, m:m + 1])
                    # fc2 + residual
                    with (
                        tc.tile_pool(name="wfc2p", bufs=3) as wfc2p,
                        tc.tile_pool(name="ps7", bufs=1,
                                     space="PSUM") as ps7,
                        tc.tile_pool(name="outp", bufs=2) as outp,
                    ):
                        pso = [ps7.tile([P, NT], F32, tag=f"fo{m}", name=f"fo{m}")
                               for m in range(NCH)]
                        for k in range(NFCH):
                            wt2 = wfc2p.tile([P, C], BF, tag="wfc2", name="wfc2")
                            nc.sync.dma_start(wt2[:],
                                              wfc2[k * P:(k + 1) * P, :])
                            for m in range(NCH):
                                nc.tensor.matmul(
                                    pso[m][:], wt2[:, m * P:(m + 1) * P],
                                    hmlp[k][:], start=(k == 0),
                                    stop=(k == NFCH - 1))
                        for m in range(NCH):
                            ot = outp.tile([P, NT], F32, tag="ot", name="ot")
                            nc.vector.scalar_tensor_tensor(
                                ot[:], pso[m][:], bias_fc2[:, m:m + 1],
                                h1[m][:], OP.add, OP.add)
                            nc.sync.dma_start(out[m * P:(m + 1) * P, :],
                                              ot[:])

    nc.compile()
    return nc


def _host_prep(x, ln1_g, ln1_b, W_attn, b_attn, W_proj, b_proj,
               ln2_g, ln2_b, W_fc, b_fc, W_fc2, b_fc2):
    """Build the 8 per-core input maps."""
    x = np.asarray(x, dtype=np.float32)
    W_attn = np.asarray(W_attn, dtype=np.float32)
    b_attn = np.asarray(b_attn, dtype=np.float32)

    wq_ = W_attn[:, 0:C].astype(BF16)
    wk_ = W_attn[:, C:2 * C].astype(BF16)
    wv_ = W_attn[:, 2 * C:3 * C].astype(BF16)
    wp_ = np.asarray(W_proj, np.float32).astype(BF16)
    wfc_ = np.asarray(W_fc, np.float32).astype(BF16)
    wfc2_ = np.asarray(W_fc2, np.float32).astype(BF16)
    shared = dict(
        wq=wq_, wk=wk_, wv=wv_, wp=wp_, wfc=wfc_, wfc2=wfc2_,
        bq8=(b_attn[0:C] / 8.0).astype(np.float32),
        bk=b_attn[C:2 * C].astype(np.float32),
        bv=b_attn[2 * C:3 * C].astype(np.float32),
        bp=np.asarray(b_proj, np.float32),
        bfc=np.asarray(b_fc, np.float32),
        bfc2=np.asarray(b_fc2, np.float32),
        g1=np.asarray(ln1_g, np.float32), b1=np.asarray(ln1_b, np.float32),
        g2=np.asarray(ln2_g, np.float32), b2=np.asarray(ln2_b, np.float32),
    )

    # causal masks per slab position s: mask[kt, k, t] = (kt*128+k <= 512s+t)
    kpos = np.arange(T).reshape(NKT, P, 1)
    tpos = np.arange(NT).reshape(1, 1, NT)
    masks = [(kpos <= 512 * s + tpos).astype(BF16) for s in range(4)]

    in_maps = []
    for c in range(NCORES):
        b, s = c // 4, c % 4
        xb = x[b]                      # [T, C]
        xo = xb[512 * s:512 * (s + 1)]  # [NT, C]
        m = dict(shared)
        m["xkv"] = np.ascontiguousarray(xb.T).astype(BF16)
        m["xq"] = np.ascontiguousarray(xo.T).astype(BF16)
        m["xres"] = np.ascontiguousarray(xo.T)
        m["maskd"] = masks[s]
        in_maps.append(m)
    return in_maps


def kernel(x, ln1_g, ln1_b, W_attn, b_attn, W_proj, b_proj,
           ln2_g, ln2_b, W_fc, b_fc, W_fc2, b_fc2):
    global _COMPILED
    from concourse.bass_utils import run_bass_kernel_spmd

    if _COMPILED is None:
        _COMPILED = _build()
    nc = _COMPILED
    in_maps = _host_prep(x, ln1_g, ln1_b, W_attn, b_attn, W_proj, b_proj,
                         ln2_g, ln2_b, W_fc, b_fc, W_fc2, b_fc2)
    res = run_bass_kernel_spmd(nc, in_maps, list(range(NCORES)))
    out = np.empty((2, T, C), dtype=np.float32)
    for c in range(NCORES):
        b, s = c // 4, c % 4
        out[b, 512 * s:512 * (s + 1), :] = res.results[c]["out"].T
    return out

